# revision 1
# baseline (speedup 1.0000x reference)
"""Trainium2 Bass kernel for vq_codebook argmin (nn_GUMSampler).

Per pixel p (4M pixels), compute d2[v] = ||z_p - vertex_v||^2 for 16 vertices
in R^15, output argmin index (int32) and min distance (f32).

Strategy (per NeuronCore, pixels sharded 8 ways):
  - Pixels are packed 8-per-PSUM-column ("groups"): PSUM row 8v+g holds
    d2 of vertex v for pixel-group g.  Full d2 is accumulated in PSUM by
    three chained matmuls sharing one 128-row output:
       mm_z :  lhsT[15g+c, 8v+g] = -2*V[v,c]     rhs = z tile     (K=120)
       mm_1 :  lhsT[g,     8v+g] = |V_v|^2       rhs = ones       (K=8)
       mm_sq:  lhsT[15g+c, 8v+g] = 1.0           rhs = z^2 tile   (K=120)
  - Vertex index is bit-packed into the 4 low mantissa bits of d2
    (d2 >= ~4 always, so a 16-ulp perturbation is ~1e-6 relative):
       packed = (bits(d2) & ~15) | v
    f32 min over packed values then yields min-d2 AND its argmin, with
    jnp.argmin's first-index tie-break (smaller v == smaller packed).
  - 16->1 min over the v rows is a partition-halving tree.  The Neuron
    compiler requires equal base partitions when both tensor_tensor inputs
    are SBUF (and 32-aligned bases for any compute read), so each level is
    (shift-copy upper half to partition 0) + (aligned tensor_tensor min).
    Shift copies are spread across ACT (32-aligned ones) and SBUF->SBUF DMA
    (sub-32 bases, which compute engines cannot read); mins run on DVE.
    ACT also pre-copies PSUM->SBUF so the pack runs in DVE 2x mode.
  - Epilogue per 4 tiles: idx = packed & 15, dmin = sqrt(packed) (the 4
    index bits perturb d2 by <= 15 ulp, ~2e-6 relative, below tolerance).
"""

import sys

sys.path.insert(0, "/opt/trn_rl_repo")

from contextlib import ExitStack

import numpy as np

import concourse.bacc as bacc
import concourse.tile as tile
from concourse import mybir
from concourse.bass_utils import run_bass_kernel_spmd

F32 = mybir.dt.float32
I32 = mybir.dt.int32

K = 16          # vertices
C = 15          # channels (K-1)
G = 8           # pixel groups per PSUM column
EFF = 1024      # pixels per iteration per group (2 PSUM banks)
N_CORES = 8
LX = LY = 2048
N_TOTAL = LX * LY
N_LOC = N_TOTAL // N_CORES          # 524288 pixels per core
N_ITERS = N_LOC // (G * EFF)        # 64
GBLK = N_LOC // G                   # 65536 pixels per group block

_CACHE = {}


def build_nc(n_iters=N_ITERS):
    n_loc = n_iters * G * EFF
    gblk = n_loc // G
    nc = bacc.Bacc("TRN2", target_bir_lowering=False, debug=False)

    z_d = nc.dram_tensor("z", [C, n_loc], F32, kind="ExternalInput")
    w1z_d = nc.dram_tensor("w1z", [C * G, 128], F32, kind="ExternalInput")
    w1o_d = nc.dram_tensor("w1o", [G, 128], F32, kind="ExternalInput")
    wsq_d = nc.dram_tensor("wsq", [C * G, 128], F32, kind="ExternalInput")
    vvec_d = nc.dram_tensor("vvec", [128, 1], I32, kind="ExternalInput")
    idx_d = nc.dram_tensor("idx", [n_loc], I32, kind="ExternalOutput")
    dmin_d = nc.dram_tensor("dmin", [n_loc], F32, kind="ExternalOutput")

    AND_MASK = -16  # 0xFFFFFFF0
    MIN = mybir.AluOpType.min

    with tile.TileContext(nc) as tc, ExitStack() as ctx:
        cpool = ctx.enter_context(tc.tile_pool(name="consts", bufs=1))
        w1z_s = cpool.tile([C * G, 128], F32)
        w1o_s = cpool.tile([G, 128], F32)
        wsq_s = cpool.tile([C * G, 128], F32)
        vvec_s = cpool.tile([128, 1], I32)
        ones_s = cpool.tile([G, EFF], F32)
        nc.sync.dma_start(w1z_s[:], w1z_d[:])
        nc.sync.dma_start(w1o_s[:], w1o_d[:])
        nc.sync.dma_start(wsq_s[:], wsq_d[:])
        nc.sync.dma_start(vvec_s[:], vvec_d[:])
        nc.vector.memset(ones_s[:], 1.0)

        zpool = ctx.enter_context(tc.tile_pool(name="z", bufs=4))
        sqpool = ctx.enter_context(tc.tile_pool(name="zsq", bufs=2))
        pspool = ctx.enter_context(tc.tile_pool(name="psum1", bufs=2, space="PSUM"))
        psbpool = ctx.enter_context(tc.tile_pool(name="psb", bufs=3))
        pkpool = ctx.enter_context(tc.tile_pool(name="pk", bufs=3))
        c1pool = ctx.enter_context(tc.tile_pool(name="c1", bufs=3))
        t1pool = ctx.enter_context(tc.tile_pool(name="t1", bufs=3))
        c2pool = ctx.enter_context(tc.tile_pool(name="c2", bufs=3))
        t2pool = ctx.enter_context(tc.tile_pool(name="t2", bufs=3))
        c3pool = ctx.enter_context(tc.tile_pool(name="c3", bufs=3))
        t3pool = ctx.enter_context(tc.tile_pool(name="t3", bufs=3))
        c4pool = ctx.enter_context(tc.tile_pool(name="c4", bufs=3))
        bscpool = ctx.enter_context(tc.tile_pool(name="bsc", bufs=2))
        dmpool = ctx.enter_context(tc.tile_pool(name="dm", bufs=2))
        idxpool = ctx.enter_context(tc.tile_pool(name="idx", bufs=2))

        idx_view = idx_d[:].rearrange(
            "(g bb j f) -> bb j g f", g=G, bb=n_iters // 2, j=4, f=512
        )
        dmin_view = dmin_d[:].rearrange(
            "(g bb j f) -> bb j g f", g=G, bb=n_iters // 2, j=4, f=512
        )

        bsc = None
        for i in range(n_iters):
            # ---- load z tile: partitions 15g+c, free = EFF pixels ----
            z_t = zpool.tile([C * G, EFF], F32)
            for g in range(G):
                off = g * gblk + i * EFF
                nc.sync.dma_start(
                    z_t[C * g : C * g + C, :], z_d[:, off : off + EFF]
                )

            # ---- z^2 on ACT ----
            zsq = sqpool.tile([C * G, EFF], F32)
            nc.scalar.square(zsq[:], z_t[:])

            # ---- full d2 into PSUM via 3 accumulating matmuls ----
            ps = pspool.tile([128, EFF], F32)
            for h in (0, 1):
                sl = slice(512 * h, 512 * h + 512)
                nc.tensor.matmul(ps[:, sl], w1z_s[:], z_t[:, sl], start=True, stop=False)
                nc.tensor.matmul(ps[:, sl], w1o_s[:], ones_s[:, sl], start=False, stop=False)
                nc.tensor.matmul(ps[:, sl], wsq_s[:], zsq[:, sl], start=False, stop=True)

            # ---- ACT copies PSUM->SBUF so the pack runs in DVE 2x mode ----
            psb = psbpool.tile([128, EFF], F32)
            nc.scalar.copy(psb[:], ps[:])

            # ---- pack: (bits(d2) & ~15) | v   [v = row >> 3] ----
            pk = pkpool.tile([128, EFF], F32)
            nc.vector.tensor_scalar(
                pk[:].bitcast(I32), psb[:].bitcast(I32), AND_MASK, vvec_s[:],
                op0=mybir.AluOpType.bitwise_and, op1=mybir.AluOpType.bitwise_or,
            )

            # ---- min tree: 128 rows (8v+g) -> 8 rows (g) ----
            c1 = c1pool.tile([64, EFF], F32)
            nc.sync.dma_start(c1[:], pk[64:128, :])
            t1 = t1pool.tile([64, EFF], F32)
            nc.vector.tensor_tensor(t1[:], pk[0:64, :], c1[:], MIN)

            c2 = c2pool.tile([32, EFF], F32)
            nc.scalar.copy(c2[:], t1[32:64, :])
            t2 = t2pool.tile([32, EFF], F32)
            nc.vector.tensor_tensor(t2[:], t1[0:32, :], c2[:], MIN)

            # partitions 16:32 / 8:16 are not 32-aligned -> compute engines
            # cannot read them; move with SBUF->SBUF DMA instead
            c3 = c3pool.tile([16, EFF], F32)
            nc.sync.dma_start(c3[:], t2[16:32, :])
            t3 = t3pool.tile([16, EFF], F32)
            nc.vector.tensor_tensor(t3[:], t2[0:16, :], c3[:], MIN)

            c4 = c4pool.tile([8, EFF], F32)
            nc.sync.dma_start(c4[:], t3[8:16, :])

            if i % 2 == 0:
                bsc = bscpool.tile([128, 512], F32)
            for h in (0, 1):
                sl = slice(512 * h, 512 * h + 512)
                j = 2 * (i % 2) + h
                nc.vector.tensor_tensor(
                    bsc[32 * j : 32 * j + 8, :], t3[0:8, sl], c4[:, sl], MIN
                )

            # ---- epilogue every 2 iterations (4 tiles of 512) ----
            if i % 2 == 1:
                b = i // 2
                # sqrt directly on packed values: the 4 index bits perturb
                # d2 by <= 15 ulp (~2e-6 relative) which is below tolerance
                dm = dmpool.tile([128, 512], F32)
                nc.scalar.sqrt(dm[:], bsc[:])
                ix = idxpool.tile([128, 512], I32)
                nc.vector.tensor_scalar(
                    ix[:], bsc[:].bitcast(I32), 15, None,
                    op0=mybir.AluOpType.bitwise_and,
                )
                dm_dst4 = dmin_view[b : b + 1].rearrange("one j g f -> (one j) g f")
                ix_dst4 = idx_view[b : b + 1].rearrange("one j g f -> (one j) g f")
                for j in range(4):
                    dm_dst = dm_dst4[j : j + 1].rearrange("one g f -> (one g) f")
                    ix_dst = ix_dst4[j : j + 1].rearrange("one g f -> (one g) f")
                    nc.sync.dma_start(dm_dst, dm[32 * j : 32 * j + 8, :])
                    nc.sync.dma_start(ix_dst, ix[32 * j : 32 * j + 8, :])

    nc.compile()
    return nc


def _weights(vertices):
    V = np.asarray(vertices, dtype=np.float32)          # (16, 15)
    vv = (V.astype(np.float64) ** 2).sum(1).astype(np.float32)
    w1z = np.zeros((C * G, 128), dtype=np.float32)
    w1o = np.zeros((G, 128), dtype=np.float32)
    wsq = np.zeros((C * G, 128), dtype=np.float32)
    for g in range(G):
        # output column 8v+g
        w1z[C * g : C * g + C, g::G] = -2.0 * V.T        # (15, 16)
        w1o[g, g::G] = vv
        wsq[C * g : C * g + C, g::G] = 1.0
    vvec = (np.arange(128, dtype=np.int32) >> 3).reshape(128, 1)
    return w1z, w1o, wsq, vvec.astype(np.int32)


def kernel(z, vertices):
    z = np.ascontiguousarray(np.asarray(z, dtype=np.float32))
    k, lx, ly = K, z.shape[1], z.shape[2]
    n = lx * ly
    z_fl = z.reshape(C, n)
    n_loc = n // N_CORES

    if "nc" not in _CACHE:
        _CACHE["nc"] = build_nc()
    nc = _CACHE["nc"]

    w1z, w1o, wsq, vvec = _weights(vertices)
    in_maps = []
    for c in range(N_CORES):
        in_maps.append(
            {
                "z": np.ascontiguousarray(z_fl[:, c * n_loc : (c + 1) * n_loc]),
                "w1z": w1z,
                "w1o": w1o,
                "wsq": wsq,
                "vvec": vvec,
            }
        )
    res = run_bass_kernel_spmd(nc, in_maps, list(range(N_CORES)))
    X = np.concatenate([res.results[c]["idx"] for c in range(N_CORES)])
    dmin = np.concatenate([res.results[c]["dmin"] for c in range(N_CORES)])
    return X.reshape(lx, ly), dmin.reshape(lx, ly)


if __name__ == "__main__":
    rng = np.random.default_rng(0)
    z = rng.standard_normal((C, 64, 64), dtype=np.float32)
    print("smoke build only")



# revision 2
# speedup vs baseline: 2.3253x; 2.3253x over previous
"""Trainium2 Bass kernel for vq_codebook argmin (nn_GUMSampler) — v3.

Per pixel p (4M pixels): d2[v] = ||z_p - vertex_v||^2 over 16 vertices in
R^15; outputs argmin index (int32) and min distance (f32).

Pixels sharded 8 ways across cores; per core N=524288 pixels in G=8 groups.

Per-core pipeline (64 rounds of 8192 pixels, FD=1024 per group-round):
  PE    d2 = w1 @ z + wsq @ z^2 into PSUM rows 8v+g   (fp16 in, f32 accum;
        vv split hi/lo over the two ones-row slots for near-f32 accuracy)
  DVE+ACT  z^2 in fp16 (free-dim split across both engines)
  ACT   evacuate PSUM -> SBUF fused with sqrt: ev = sqrt(d2) (monotonic,
        so min/argmin are preserved and no separate sqrt pass is needed)
  DVE   pack candidate index v into the low 4 mantissa bits, touching only
        the LOW uint16 of each f32 (TS 2x_2P on half the elements)
  PE    transpose packed f32 (bit-exact) -> PSUM pixel-major
  DVE   one segmented strided tensor_reduce(min) over the 16 candidates
        (reads PSUM directly; packed min = value + argmin in one word)
The packed f32 min IS the output word: its value is dmin (sqrt is fused
into the PSUM evacuation, low-bit perturbation ~1e-6) and its low 4 bits
are the argmin index, extracted host-side. One output DMA at the end.
"""

import sys

sys.path.insert(0, "/opt/trn_rl_repo")

from contextlib import ExitStack

import numpy as np

import concourse.bacc as bacc
import concourse.tile as tile
from concourse import mybir
from concourse.bass_utils import run_bass_kernel_spmd

F32 = mybir.dt.float32
F16 = mybir.dt.float16
BF16 = mybir.dt.bfloat16
I32 = mybir.dt.int32
I8 = mybir.dt.int8
U16 = mybir.dt.uint16
MIN = mybir.AluOpType.min

K = 16            # vertices
C = 15            # channels
G = 8             # pixel groups (PSUM row = 8v+g)
RFD = 1024        # pixels per group per round
N_CORES = 8
LX = LY = 2048
N_TOTAL = LX * LY
N_LOC = N_TOTAL // N_CORES          # 524288
N_ROUNDS = N_LOC // (G * RFD)       # 64
GBLK = N_LOC // G                   # 65536 pixels per group
ZSQ_DVE = 320                       # zsq slab columns done on DVE (rest on ACT)
OPR = RFD // 16                     # output columns per round

_CACHE = {}


def build_nc(n_rounds=N_ROUNDS):
    gblk = n_rounds * RFD
    nc = bacc.Bacc("TRN2", target_bir_lowering=False, debug=False)

    zt_d = nc.dram_tensor("zt", [128, gblk], F16, kind="ExternalInput")
    w1_d = nc.dram_tensor("w1", [128, 128], F16, kind="ExternalInput")
    wsq_d = nc.dram_tensor("wsq", [128, 128], F16, kind="ExternalInput")
    ident_d = nc.dram_tensor("ident", [128, 128], F32, kind="ExternalInput")
    vvec_d = nc.dram_tensor("vvec", [128, 1], U16, kind="ExternalInput")
    dm_d = nc.dram_tensor("dm_o", [128, n_rounds * OPR], F32, kind="ExternalOutput")

    with tile.TileContext(nc) as tc, ExitStack() as ctx:
        cpool = ctx.enter_context(tc.tile_pool(name="consts", bufs=1))
        w1_s = cpool.tile([128, 128], F16)
        wsq_s = cpool.tile([128, 128], F16)
        ident_s = cpool.tile([128, 128], F32)
        vvec_s = cpool.tile([128, 1], U16)
        nc.sync.dma_start(w1_s[:], w1_d[:])
        nc.sync.dma_start(wsq_s[:], wsq_d[:])
        nc.sync.dma_start(ident_s[:], ident_d[:])
        nc.sync.dma_start(vvec_s[:], vvec_d[:])

        ztpool = ctx.enter_context(tc.tile_pool(name="zt", bufs=1))
        zt = ztpool.tile([128, gblk], F16)
        # load z in graded chunks (small first) so compute starts early
        if n_rounds >= 16:
            bounds = [0, 2, 6, 14]
            step = (n_rounds - 14) // 5
            for i in range(1, 5):
                bounds.append(14 + step * i)
            bounds.append(n_rounds)
        else:
            bounds = list(range(n_rounds + 1))
        for ch in range(len(bounds) - 1):
            lo, hi = bounds[ch] * RFD, bounds[ch + 1] * RFD
            if hi > lo:
                nc.sync.dma_start(zt[:, lo:hi], zt_d[:, lo:hi])

        zsqpool = ctx.enter_context(tc.tile_pool(name="zsq", bufs=3))
        pspool = ctx.enter_context(tc.tile_pool(name="d2ps", bufs=2, space="PSUM"))
        epool = ctx.enter_context(tc.tile_pool(name="evac", bufs=4))
        tpool = ctx.enter_context(tc.tile_pool(name="tps", bufs=2, space="PSUM"))
        pmpool = ctx.enter_context(tc.tile_pool(name="pm", bufs=4))
        opool = ctx.enter_context(tc.tile_pool(name="outs", bufs=1))
        dm_acc = opool.tile([128, n_rounds * OPR], F32)

        SLAB = 2 * RFD
        for s in range(n_rounds // 2):
            zslab = zt[:, s * SLAB : (s + 1) * SLAB]

            # ---- z^2 fp16 for the whole slab, split across DVE and ACT ----
            zsq = zsqpool.tile([128, SLAB], F16)
            nc.vector.tensor_tensor(
                zsq[:, :ZSQ_DVE], zslab[:, :ZSQ_DVE], zslab[:, :ZSQ_DVE],
                mybir.AluOpType.mult,
            )
            nc.scalar.square(zsq[:, ZSQ_DVE:], zslab[:, ZSQ_DVE:])

            ev = epool.tile([128, SLAB], F32)
            for half in range(2):
                r = 2 * s + half
                zs = zslab[:, half * RFD : (half + 1) * RFD]
                evh = ev[:, half * RFD : (half + 1) * RFD]

                # ---- d2 into PSUM: rows 8v+g ----
                ps = pspool.tile([128, RFD], F32)
                for h in range(RFD // 512):
                    sl = slice(512 * h, 512 * h + 512)
                    nc.tensor.matmul(ps[:, sl], w1_s[:], zs[:, sl], start=True, stop=False)
                for h in range(RFD // 512):
                    sl = slice(512 * h, 512 * h + 512)
                    nc.tensor.matmul(
                        ps[:, sl], wsq_s[:], zsq[:, half * RFD + sl.start : half * RFD + sl.stop],
                        start=False, stop=True,
                    )

                # ---- evacuate to SBUF fused with sqrt: ev = sqrt(d2) ----
                nc.scalar.sqrt(evh, ps[:])

            # ---- pack index into low 4 mantissa bits, whole slab at once ----
            lo = ev[:].bitcast(U16).rearrange("p (n two) -> p n two", two=2)[:, :, 0]
            nc.vector.tensor_scalar(
                lo, lo, -16, vvec_s[:],
                op0=mybir.AluOpType.bitwise_and, op1=mybir.AluOpType.bitwise_or,
            )

            for half in range(2):
                r = 2 * s + half
                evh = ev[:, half * RFD : (half + 1) * RFD]

                # ---- transpose to pixel-major PSUM (bit-exact f32) ----
                tp = tpool.tile([128, RFD], F32)
                for b in range(RFD // 128):
                    sl = slice(128 * b, 128 * b + 128)
                    nc.tensor.transpose(tp[:, sl], evh[:, sl], ident_s[:])

                # ---- segmented strided min-reduce -> packed dmin ----
                osl = slice(r * OPR, r * OPR + OPR)
                vview = tp[:].rearrange("p (b v g) -> p b g v", b=RFD // 128, v=K, g=G)
                nc.vector.tensor_reduce(dm_acc[:, osl], vview, mybir.AxisListType.X, MIN)

            # stream out the finished head once most rounds are done, so only
            # the tail DMA sits past the last round
            if n_rounds >= 16 and r == n_rounds - 1 - n_rounds // 8:
                mid_out = (r + 1) * OPR
                nc.sync.dma_start(dm_d[:, :mid_out], dm_acc[:, :mid_out])
        mid_out = (n_rounds - n_rounds // 8) * OPR if n_rounds >= 16 else 0
        nc.sync.dma_start(dm_d[:, mid_out:], dm_acc[:, mid_out:])

    nc.compile()
    return nc


def _weights(vertices):
    V = np.asarray(vertices, dtype=np.float32)              # (16, 15)
    vv = (V.astype(np.float64) ** 2).sum(1).astype(np.float32)
    vv_hi = vv.astype(np.float16)
    vv_lo = (vv - vv_hi.astype(np.float32)).astype(np.float16)
    w1 = np.zeros((128, 128), dtype=np.float16)
    wsq = np.zeros((128, 128), dtype=np.float16)
    for g in range(G):
        for v in range(K):
            col = 8 * v + g
            w1[16 * g : 16 * g + 15, col] = (-2.0 * V[v]).astype(np.float16)
            w1[16 * g + 15, col] = vv_hi[v]
            wsq[16 * g : 16 * g + 15, col] = 1.0
            wsq[16 * g + 15, col] = vv_lo[v]
    ident = np.eye(128, dtype=np.float32)
    vvec = (np.arange(128, dtype=np.uint16) >> 3).reshape(128, 1).astype(np.uint16)
    return w1, wsq, ident, vvec


def _prep_zt(z_core):
    # z_core: (15, N_loc) f32 -> [128, GBLK] f16 with ones rows at 16g+15
    n = z_core.shape[1]
    gblk = n // G
    zt = np.empty((128, gblk), dtype=np.float16)
    zg = z_core.reshape(C, G, gblk)
    for g in range(G):
        zt[16 * g : 16 * g + 15] = zg[:, g].astype(np.float16)
        zt[16 * g + 15] = np.float16(1.0)
    return zt


def _unscramble(arr, n_rounds):
    # arr [128, n_rounds*64] laid (q, (r, b, g)) -> per-group-major flat pixels
    a = np.asarray(arr).reshape(128, n_rounds, -1, 8)      # q, r, b, g
    a = a.transpose(3, 1, 2, 0)                            # g, r, b, q
    return np.ascontiguousarray(a).reshape(-1)             # g-major flat


def kernel(z, vertices):
    z = np.ascontiguousarray(np.asarray(z, dtype=np.float32))
    lx, ly = z.shape[1], z.shape[2]
    n = lx * ly
    z_fl = z.reshape(C, n)
    n_loc = n // N_CORES

    if "nc" not in _CACHE:
        _CACHE["nc"] = build_nc()
    nc = _CACHE["nc"]

    w1, wsq, ident, vvec = _weights(vertices)
    in_maps = []
    for c in range(N_CORES):
        zt = _prep_zt(z_fl[:, c * n_loc : (c + 1) * n_loc])
        in_maps.append(
            {"zt": zt, "w1": w1, "wsq": wsq, "ident": ident, "vvec": vvec}
        )
    res = run_bass_kernel_spmd(nc, in_maps, list(range(N_CORES)))

    Xs, Ds = [], []
    for c in range(N_CORES):
        dmc = _unscramble(np.asarray(res.results[c]["dm_o"]), N_ROUNDS)
        Xs.append((dmc.view(np.int32) & 15).astype(np.int32))
        Ds.append(dmc.astype(np.float32))
    X = np.concatenate(Xs).reshape(lx, ly)
    D = np.concatenate(Ds).reshape(lx, ly)
    return X, D


if __name__ == "__main__":
    print("module ok")
